# revision 20
# baseline (speedup 1.0000x reference)
"""DampedIMEX1Layer forward on 8 Trainium2 NeuronCores via a Bass/Tile kernel.

Sharding: data-parallel over batch (2 sequences per core). Per core, one
hand-written NEFF computes  y = Re(C @ x) + D*u  where x is the IMEX1
2x2-per-mode linear recurrence state, via:
  - input projection GEMM (tensor engine, bf16)
  - block-clipped FIR over W=16 lags in the projected (y) domain (tensor eng)
  - per-block boundary states + hierarchical 2x2 affine scan (vector engine)
  - per-phase carry projection GEMM + evict (tensor + vector engines)
Input is shipped int8 (per-channel scales folded into weights), output bf16.
"""
import hashlib
from contextlib import ExitStack

import numpy as np
import ml_dtypes

EPS = 1e-6
Bsz, L, H, P = 16, 4096, 128, 256
NCORES, BPC = 8, 2
W = 16
NB = L // W
G2 = 16
NG = NB // G2
NSEQ = BPC
NTOK = NSEQ * L
P2 = 2 * P
NPB = P2 // 128
SEQW = W + L
BUW = NSEQ * SEQW

# ----------------------------------------------------------------- tables --

def _mode_params(A_diag, G_diag, dt):
    f64 = np.float64
    dt_s = 1.0 / (1.0 + np.exp(-dt.astype(f64)))
    A = np.maximum(A_diag.astype(f64), 0.0)
    G = np.maximum(G_diag.astype(f64), 0.0)
    root = np.sqrt(1.0 + dt_s * G)
    denom = np.maximum(dt_s ** 2, EPS)
    A_low = (2.0 + dt_s * G - 2.0 * root) / denom
    A_high = (2.0 + dt_s * G + 2.0 * root) / denom
    A = A_low + np.maximum(A - A_low, 0) - np.maximum(A - A_high, 0)
    S = 1.0 / (1.0 + dt_s * G)
    return S, -A * dt_s * S, dt_s * S, 1.0 - A * dt_s * dt_s * S, \
        dt_s * S, dt_s * dt_s * S


def _mat_pows_of(Mtup, n):
    a, b, c, d = Mtup
    A = np.zeros((n + 1, P)); B = np.zeros((n + 1, P))
    C = np.zeros((n + 1, P)); D = np.zeros((n + 1, P))
    A[0] = 1.0; D[0] = 1.0
    for l in range(1, n + 1):
        A[l] = a * A[l - 1] + b * C[l - 1]
        B[l] = a * B[l - 1] + b * D[l - 1]
        C[l] = c * A[l - 1] + d * C[l - 1]
        D[l] = c * B[l - 1] + d * D[l - 1]
    return A, B, C, D


def _dup(x):
    return np.concatenate([x, x], axis=-1)


def _build_tables(A_diag, G_diag, dt, B, C, D, uscale):
    mA, mB, mC, mD, c1, c2 = _mode_params(A_diag, G_diag, dt)
    hA, hB, hC, hD = _mat_pows_of((mA, mB, mC, mD), W + 1)
    wZ = hA[:W] * c1 + hB[:W] * c2
    wX = hC[:W] * c1 + hD[:W] * c2
    M16P = _mat_pows_of((hA[W], hB[W], hC[W], hD[W]), G2)
    M256P = _mat_pows_of(tuple(x[G2] for x in M16P), NG)
    hCj = hC[1:W + 1]; hDj = hD[1:W + 1]
    B0T = (B[:, :, 0].T * uscale[:, None])
    B1T = (B[:, :, 1].T * uscale[:, None])
    Cp = np.concatenate([C[:, :, 0], -C[:, :, 1]], axis=1)
    Gf = Cp[None, :, :] * _dup(wX)[:, None, :]
    f32 = lambda x: np.ascontiguousarray(x, np.float32)
    return dict(
        WZr=f32(_dup(wZ[::-1])), WXr=f32(_dup(wX[::-1])),
        M16P=[f32(_dup(x)) for x in M16P],
        M256P=[f32(_dup(x)) for x in M256P],
        hCj=f32(_dup(hCj)), hDj=f32(_dup(hDj)),
        BT=f32(np.concatenate([B0T, B1T], axis=1)),
        Cp=f32(Cp), Gf=f32(Gf), Dp=f32(D * uscale),
    )


def _wb_layout():
    t = [("BT", pb, 128, 128) for pb in range(NPB)]
    t += [("Gf", (d, pb), 128, 128) for d in range(W) for pb in range(NPB)]
    t += [("Gz", (j, pb), 128, 128) for j in range(W) for pb in range(NPB)]
    t += [("Gx", (j, pb), 128, 128) for j in range(W) for pb in range(NPB)]
    return t


def _wf_layout():
    t = [("Cp", pb, 128, 128) for pb in range(NPB)]
    t += [("WZr", pb, 128, W) for pb in range(NPB)]
    t += [("WXr", pb, 128, W) for pb in range(NPB)]
    t += [("M16P", pb, 128, 4 * (G2 + 1)) for pb in range(NPB)]
    t += [("M256P", pb, 128, 4 * (NG + 1)) for pb in range(NPB)]
    t += [("hCjc", pb, 128, W) for pb in range(NPB)]
    t += [("hDjc", pb, 128, W) for pb in range(NPB)]
    t += [("Dp", 0, 128, 1)]
    return t


def _pack_weights(T):
    wb_parts, wf_parts = [], []
    Gz = T["Cp"][None] * T["hCj"][:, None, :]      # (W, H, 2P)
    Gx = T["Cp"][None] * T["hDj"][:, None, :]
    for name, key, r, c in _wb_layout():
        if name == "BT":
            a = T["BT"][:, key * 128:(key + 1) * 128]
        elif name == "Gf":
            d, pb = key
            a = T["Gf"][d][:, pb * 128:(pb + 1) * 128].T
        elif name == "Gz":
            j, pb = key
            a = Gz[j][:, pb * 128:(pb + 1) * 128].T
        else:
            j, pb = key
            a = Gx[j][:, pb * 128:(pb + 1) * 128].T
        wb_parts.append(np.ascontiguousarray(a, np.float32))
    for name, key, r, c in _wf_layout():
        sl = slice(key * 128, (key + 1) * 128)
        if name == "Cp":
            a = T["Cp"][:, sl].T
        elif name == "WZr":
            a = T["WZr"][:, sl].T
        elif name == "WXr":
            a = T["WXr"][:, sl].T
        elif name == "M16P":
            a = np.stack([T["M16P"][e][l][sl]
                          for l in range(G2 + 1) for e in range(4)], axis=1)
        elif name == "M256P":
            a = np.stack([T["M256P"][e][l][sl]
                          for l in range(NG + 1) for e in range(4)], axis=1)
        elif name == "hCjc":
            a = T["hCj"][:, sl].T
        elif name == "hDjc":
            a = T["hDj"][:, sl].T
        elif name == "Dp":
            a = T["Dp"][:, None]
        wf_parts.append(np.ascontiguousarray(a, np.float32))
    wb = np.hstack(wb_parts).astype(ml_dtypes.bfloat16)
    wf = np.hstack(wf_parts).astype(np.float32)
    return np.ascontiguousarray(wb), np.ascontiguousarray(wf)

# ------------------------------------------------------------ bass kernel --

def _build_nc():
    import concourse.bacc as bacc
    import concourse.mybir as mybir
    import concourse.tile as tile

    F32, BF16, I8 = mybir.dt.float32, mybir.dt.bfloat16, mybir.dt.int8
    MULT, ADD = mybir.AluOpType.mult, mybir.AluOpType.add
    COPY = mybir.ActivationFunctionType.Copy

    nc = bacc.Bacc("TRN2", target_bir_lowering=False, debug=False,
                   num_devices=1)
    u8 = nc.dram_tensor("u8", (H, NTOK), I8, kind="ExternalInput").ap()
    nwb = sum(c for _, _, r, c in _wb_layout())
    nwf = sum(c for _, _, r, c in _wf_layout())
    wb = nc.dram_tensor("wb", (H, nwb), BF16, kind="ExternalInput").ap()
    wf = nc.dram_tensor("wf", (H, nwf), F32, kind="ExternalInput").ap()
    y16 = nc.dram_tensor("y16", (H, NTOK), BF16, kind="ExternalOutput").ap()

    with tile.TileContext(nc) as tc, ExitStack() as ctx:
        wpool = ctx.enter_context(tc.tile_pool(name="w", bufs=1))
        dpool = ctx.enter_context(tc.tile_pool(name="data", bufs=1))
        spool = ctx.enter_context(tc.tile_pool(name="state", bufs=1))
        g1ps = ctx.enter_context(tc.tile_pool(name="g1", bufs=2, space="PSUM"))
        yjps = ctx.enter_context(tc.tile_pool(name="yj", bufs=2, space="PSUM"))

        # u8 first in the SWDGE queue (first consumer), then BT (GEMM1
        # stationaries, 512 cols), then the f32 tables, then the big
        # Gf/Gz/Gx tail which is only needed once the FIR phase starts.
        u8s = dpool.tile([H, NTOK], I8, tag="u8s", name="u8s")
        nc.gpsimd.dma_start(u8s[:], u8[:])
        wbt = wpool.tile([H, nwb], BF16, tag="wbt", name="wbt")
        nc.gpsimd.dma_start(wbt[:, 0:512], wb[:, 0:512])
        wft = wpool.tile([H, nwf], F32, tag="wft", name="wft")
        nc.gpsimd.dma_start(wft[:], wf[:])
        nc.gpsimd.dma_start(wbt[:, 512:], wb[:, 512:])
        wt = {}
        off = 0
        for name, key, r, c in _wb_layout():
            wt[(name, key)] = wbt[:, off:off + c]; off += c
        off = 0
        for name, key, r, c in _wf_layout():
            wt[(name, key)] = wft[:, off:off + c]; off += c

        # Bu in PHASE-MAJOR layout: col = j*(NSEQ*NB) + s*NB + i for token
        # l = W*i + j of sequence s.  Every FIR read is then contiguous.
        y = dpool.tile([H, NTOK], BF16, tag="y", name="yt")
        Dp = wt[("Dp", 0)]
        Bu = [dpool.tile([128, W * NSEQ * NB], BF16, tag=f"bu{pb}",
                         name=f"bu{pb}") for pb in range(NPB)]
        BuJ = [Bu[pb].rearrange("p (j s i) -> p s i j", j=W, s=NSEQ, i=NB)
               for pb in range(NPB)]
        upool = ctx.enter_context(tc.tile_pool(name="u8c", bufs=3))
        NBT = 512 // W                      # blocks per 512-token tile (32)
        for s in range(NSEQ):
            for t in range(L // 512):
                c0 = s * L + t * 512
                rhs = upool.tile([H, 512], BF16, tag="u16c", name="u16c")
                nc.scalar.activation(rhs[:], u8s[:, c0:c0 + 512], COPY)
                nc.scalar.activation(y[:, c0:c0 + 512], rhs[:], COPY,
                                     scale=Dp[:, 0:1])
                for pb in range(NPB):
                    ps = g1ps.tile([128, 512], F32, tag="g1", name="g1ps")
                    nc.tensor.matmul(ps[:], wt[("BT", pb)][:], rhs[:],
                                     start=True, stop=True)
                    psv = ps.rearrange("p (i j) -> p i j", i=NBT, j=W)
                    dst = BuJ[pb][:, s, t * NBT:(t + 1) * NBT, :]
                    if pb < 2:
                        nc.scalar.activation(dst, psv[:], COPY)
                    else:
                        nc.vector.tensor_copy(dst, psv[:])

        zloc = [spool.tile([128, NSEQ * NB], F32, tag=f"zl{pb}",
                           name=f"zl{pb}") for pb in range(NPB)]
        xloc = [spool.tile([128, NSEQ * NB], F32, tag=f"xl{pb}",
                           name=f"xl{pb}") for pb in range(NPB)]
        NSB = NSEQ * NB
        for pb in range(NPB):
            for dst, wrow, eng in ((zloc, "WZr", nc.vector),
                                   (xloc, "WXr", nc.vector)):
                dv = dst[pb][:]
                w_ = wt[(wrow, pb)]
                for k in range(W):
                    src = Bu[pb][:, k * NSB:(k + 1) * NSB]
                    if k == 0:
                        eng.tensor_scalar(dv, src, w_[:, 0:1], None, MULT)
                    else:
                        eng.scalar_tensor_tensor(
                            dv, src, w_[:, k:k + 1], dv, MULT, ADD)

        Pz = [spool.tile([128, NSEQ * NG * (G2 + 1)], F32, tag=f"pz{pb}",
                         name=f"pz{pb}") for pb in range(NPB)]
        Px = [spool.tile([128, NSEQ * NG * (G2 + 1)], F32, tag=f"px{pb}",
                         name=f"px{pb}") for pb in range(NPB)]
        for pb in range(NPB):
            Pzv = Pz[pb].rearrange("p (s g i) -> p s g i",
                                   s=NSEQ, g=NG, i=G2 + 1)
            Pxv = Px[pb].rearrange("p (s g i) -> p s g i",
                                   s=NSEQ, g=NG, i=G2 + 1)
            zlg = zloc[pb].rearrange("p (s g i) -> p s g i",
                                     s=NSEQ, g=NG, i=G2)
            xlg = xloc[pb].rearrange("p (s g i) -> p s g i",
                                     s=NSEQ, g=NG, i=G2)
            nc.vector.memset(Pzv[:, :, :, 0:1], 0.0)
            nc.vector.memset(Pxv[:, :, :, 0:1], 0.0)
            nc.vector.tensor_copy(Pzv[:, :, :, 1:], zlg[:])
            nc.vector.tensor_copy(Pxv[:, :, :, 1:], xlg[:])
            mt = wt[("M16P", pb)]
            for lag in range(1, G2):
                A_ = mt[:, 4 * lag + 0: 4 * lag + 1]
                B_ = mt[:, 4 * lag + 1: 4 * lag + 2]
                C_ = mt[:, 4 * lag + 2: 4 * lag + 3]
                D_ = mt[:, 4 * lag + 3: 4 * lag + 4]
                zsrc = zlg[:, :, :, 0:G2 - lag]
                xsrc = xlg[:, :, :, 0:G2 - lag]
                zdst = Pzv[:, :, :, 1 + lag:]
                xdst = Pxv[:, :, :, 1 + lag:]
                nc.vector.scalar_tensor_tensor(zdst, zsrc, A_, zdst, MULT, ADD)
                nc.vector.scalar_tensor_tensor(zdst, xsrc, B_, zdst, MULT, ADD)
                nc.vector.scalar_tensor_tensor(xdst, zsrc, C_, xdst, MULT, ADD)
                nc.vector.scalar_tensor_tensor(xdst, xsrc, D_, xdst, MULT, ADD)

        z2 = [spool.tile([128, NSEQ * (NG + 1)], F32, tag=f"z2{pb}",
                         name=f"z2{pb}") for pb in range(NPB)]
        x2 = [spool.tile([128, NSEQ * (NG + 1)], F32, tag=f"x2{pb}",
                         name=f"x2{pb}") for pb in range(NPB)]
        for pb in range(NPB):
            z2v = z2[pb].rearrange("p (s c) -> p s c", s=NSEQ)
            x2v = x2[pb].rearrange("p (s c) -> p s c", s=NSEQ)
            Pzv = Pz[pb].rearrange("p (s g i) -> p s g i",
                                   s=NSEQ, g=NG, i=G2 + 1)
            Pxv = Px[pb].rearrange("p (s g i) -> p s g i",
                                   s=NSEQ, g=NG, i=G2 + 1)
            s2z = Pzv[:, :, :, G2:G2 + 1]
            s2x = Pxv[:, :, :, G2:G2 + 1]
            nc.vector.memset(z2v[:, :, 0:1], 0.0)
            nc.vector.memset(x2v[:, :, 0:1], 0.0)
            z2g = z2[pb].rearrange("p (s c one) -> p s c one",
                                   s=NSEQ, c=NG + 1, one=1)
            x2g = x2[pb].rearrange("p (s c one) -> p s c one",
                                   s=NSEQ, c=NG + 1, one=1)
            nc.vector.tensor_copy(z2g[:, :, 1:, :], s2z[:])
            nc.vector.tensor_copy(x2g[:, :, 1:, :], s2x[:])
            mt = wt[("M256P", pb)]
            for lag in range(1, NG):
                A_ = mt[:, 4 * lag + 0: 4 * lag + 1]
                B_ = mt[:, 4 * lag + 1: 4 * lag + 2]
                C_ = mt[:, 4 * lag + 2: 4 * lag + 3]
                D_ = mt[:, 4 * lag + 3: 4 * lag + 4]
                zsrc = s2z[:, :, 0:NG - lag, :]
                xsrc = s2x[:, :, 0:NG - lag, :]
                zdst = z2g[:, :, 1 + lag:, :]
                xdst = x2g[:, :, 1 + lag:, :]
                nc.vector.scalar_tensor_tensor(zdst, zsrc, A_, zdst, MULT, ADD)
                nc.vector.scalar_tensor_tensor(zdst, xsrc, B_, zdst, MULT, ADD)
                nc.vector.scalar_tensor_tensor(xdst, zsrc, C_, xdst, MULT, ADD)
                nc.vector.scalar_tensor_tensor(xdst, xsrc, D_, xdst, MULT, ADD)

        zin = [spool.tile([128, NSEQ * NB], BF16, tag=f"zi{pb}",
                          name=f"zi{pb}") for pb in range(NPB)]
        xin = [spool.tile([128, NSEQ * NB], BF16, tag=f"xi{pb}",
                          name=f"xi{pb}") for pb in range(NPB)]
        for pb in range(NPB):
            ziv = zin[pb].rearrange("p (s g i) -> p s g i",
                                    s=NSEQ, g=NG, i=G2)
            xiv = xin[pb].rearrange("p (s g i) -> p s g i",
                                    s=NSEQ, g=NG, i=G2)
            Pzv = Pz[pb].rearrange("p (s g i) -> p s g i",
                                   s=NSEQ, g=NG, i=G2 + 1)
            Pxv = Px[pb].rearrange("p (s g i) -> p s g i",
                                   s=NSEQ, g=NG, i=G2 + 1)
            z2g = z2[pb].rearrange("p (s c one) -> p s c one",
                                   s=NSEQ, c=NG + 1, one=1)
            x2g = x2[pb].rearrange("p (s c one) -> p s c one",
                                   s=NSEQ, c=NG + 1, one=1)
            z2in = z2g[:, :, 0:NG, :]
            x2in = x2g[:, :, 0:NG, :]
            mt = wt[("M16P", pb)]
            for i in range(G2):
                A_ = mt[:, 4 * i + 0: 4 * i + 1]
                B_ = mt[:, 4 * i + 1: 4 * i + 2]
                C_ = mt[:, 4 * i + 2: 4 * i + 3]
                D_ = mt[:, 4 * i + 3: 4 * i + 4]
                zo = ziv[:, :, :, i:i + 1]
                xo = xiv[:, :, :, i:i + 1]
                pze = Pzv[:, :, :, i:i + 1]
                pxe = Pxv[:, :, :, i:i + 1]
                nc.vector.scalar_tensor_tensor(zo, z2in, A_, pze, MULT, ADD)
                nc.vector.scalar_tensor_tensor(zo, x2in, B_, zo, MULT, ADD)
                nc.vector.scalar_tensor_tensor(xo, z2in, C_, pxe, MULT, ADD)
                nc.vector.scalar_tensor_tensor(xo, x2in, D_, xo, MULT, ADD)

        yv = y.rearrange("p (s b w) -> p s b w", s=NSEQ, b=NB, w=W)
        for j in range(W):
            ps = yjps.tile([128, 512], F32, tag="yj", name="yjps")
            nmm = 4 * (j + 1) + 8
            k = 0
            for d in range(j + 1):
                for pb in range(NPB):
                    rhs = Bu[pb][:, (j - d) * NSB:(j - d + 1) * NSB]
                    nc.tensor.matmul(ps[:], wt[("Gf", (d, pb))][:], rhs,
                                     start=(k == 0), stop=(k == nmm - 1))
                    k += 1
            for pb in range(NPB):
                nc.tensor.matmul(ps[:], wt[("Gz", (j, pb))][:], zin[pb][:],
                                 start=(k == 0), stop=(k == nmm - 1))
                k += 1
                nc.tensor.matmul(ps[:], wt[("Gx", (j, pb))][:], xin[pb][:],
                                 start=(k == 0), stop=(k == nmm - 1))
                k += 1
            psv = ps.rearrange("p (s i one) -> p s i one",
                               s=NSEQ, i=NB, one=1)
            ysl = yv[:, :, :, j:j + 1]
            nc.vector.scalar_tensor_tensor(ysl, psv[:], 1.0, ysl, MULT, ADD)

        nc.sync.dma_start(y16[:], y[:])

    nc.compile()
    return nc

# --------------------------------------------------------------- runtime --

_STATE = {}


def _get_runner():
    """Build nc + cached jitted SPMD executor (once per process)."""
    if "runner" in _STATE:
        return _STATE["runner"]
    import jax
    import jax.numpy as jnp
    from jax.sharding import Mesh, PartitionSpec, NamedSharding
    from jax.experimental.shard_map import shard_map
    from concourse import bass2jax

    bass2jax.install_neuronx_cc_hook()
    nc = _build_nc()

    pname = (nc.partition_id_tensor.name
             if nc.partition_id_tensor is not None else None)
    in_names, out_names, out_avals = [], [], []
    import concourse.mybir as mybir
    for alloc in nc.m.functions[0].allocations:
        if not isinstance(alloc, mybir.MemoryLocationSet):
            continue
        nm = alloc.memorylocations[0].name
        if alloc.kind == "ExternalInput":
            if nm != pname:
                in_names.append(nm)
        elif alloc.kind == "ExternalOutput":
            out_names.append(nm)
            out_avals.append(jax.core.ShapedArray(
                tuple(alloc.tensor_shape), mybir.dt.np(alloc.dtype)))
    n_params = len(in_names)
    all_names = in_names + out_names
    if pname is not None:
        all_names = all_names + [pname]

    def _body(*args):
        operands = list(args)
        if pname is not None:
            operands.append(bass2jax.partition_id_tensor())
        outs = bass2jax._bass_exec_p.bind(
            *operands, out_avals=tuple(out_avals), in_names=tuple(all_names),
            out_names=tuple(out_names), lowering_input_output_aliases=(),
            sim_require_finite=True, sim_require_nnan=True, nc=nc)
        return tuple(outs)

    devices = jax.devices()[:NCORES]
    mesh = Mesh(np.asarray(devices), ("core",))
    sharding = NamedSharding(mesh, PartitionSpec("core"))
    n_out = len(out_names)
    donate = tuple(range(n_params, n_params + n_out))
    sharded = jax.jit(
        shard_map(_body, mesh=mesh,
                  in_specs=(PartitionSpec("core"),) * (n_params + n_out),
                  out_specs=(PartitionSpec("core"),) * n_out,
                  check_rep=False),
        donate_argnums=donate, keep_unused=True)
    zeros = jax.jit(
        lambda: jnp.zeros((NCORES * H, NTOK), jnp.bfloat16),
        out_shardings=sharding)

    runner = dict(sharded=sharded, zeros=zeros, sharding=sharding,
                  in_names=in_names, jax=jax)
    _STATE["runner"] = runner
    return runner


def kernel(input_sequence, A_diag, G_diag, dt, B, C, D):
    import jax
    r = _get_runner()

    u = np.asarray(input_sequence, np.float32)
    A_diag = np.asarray(A_diag); G_diag = np.asarray(G_diag)
    dt = np.asarray(dt); B = np.asarray(B); C = np.asarray(C)
    D = np.asarray(D)

    uscale = (np.abs(u).max(axis=(0, 1)) / 127.0).astype(np.float32)
    key = hashlib.md5(b"".join(x.tobytes() for x in
                               (A_diag, G_diag, dt, B, C, D, uscale)))
    key = key.hexdigest()
    if _STATE.get("wkey") != key:
        T = _build_tables(A_diag, G_diag, dt, B, C, D, uscale)
        wb, wf = _pack_weights(T)
        _STATE["wb_dev"] = jax.device_put(
            np.broadcast_to(wb, (NCORES,) + wb.shape).reshape(
                NCORES * wb.shape[0], wb.shape[1]), r["sharding"])
        _STATE["wf_dev"] = jax.device_put(
            np.broadcast_to(wf, (NCORES,) + wf.shape).reshape(
                NCORES * wf.shape[0], wf.shape[1]), r["sharding"])
        _STATE["wkey"] = key

    u8 = np.clip(np.rint(u * (1.0 / uscale)), -127, 127).astype(np.int8)
    u8g = np.ascontiguousarray(
        u8.reshape(NCORES, NSEQ, L, H).transpose(0, 3, 1, 2).reshape(
            NCORES * H, NTOK))

    args = {"u8": u8g, "wb": _STATE["wb_dev"], "wf": _STATE["wf_dev"]}
    ins = [args[n] for n in r["in_names"]]
    (out,) = r["sharded"](*ins, r["zeros"]())
    y = np.asarray(out).astype(np.float32)
    y = y.reshape(NCORES, H, NSEQ, L).transpose(0, 2, 3, 1).reshape(
        Bsz, L, H)
    return np.ascontiguousarray(y)
